# revision 7
# baseline (speedup 1.0000x reference)
"""Causal self-attention (B=2, T=4096, C=768, NH=12) on 8 trn2 cores.

Sharding: data-parallel over batch (2) x tensor-parallel over heads (12 -> 3
heads/core).  Core c handles batch c//4, heads 3*(c%4) .. 3*(c%4)+2.  Each
core computes qkv for its heads, causal attention, and its partial of the
output projection; a 4-core ReduceScatter per 1024-row chunk reduces the
partials, each core writes its 256-row shard of every chunk, and the host
reassembles the stripes.

Compute dtype: bf16 operands, fp32 PSUM accumulation.  Softmax skips the
row-max subtraction (scores are ~N(0,1); exp never overflows) and gets the
denominator from a ones-column appended to V.

Engine budget: PE matmuls; ACT exp; DVE casts/copies/normalize; GpSimd
causal masks + broadcast-DMA issue; Sync DMAs + collectives (collectives are
last in sync's stream per chunk, so their completion waits stall nothing).
"""

import sys

if "/opt/trn_rl_repo" not in sys.path:
    sys.path.insert(0, "/opt/trn_rl_repo")

import numpy as np

B, T, C = 2, 4096, 768
NH, HD = 12, 64
N_CORES = 8
HPC = 3  # heads per core
TB = 512  # q block size
KT = 128  # kv tile size
NQB = T // TB  # 8 q blocks
NTT = T // KT  # 32 t tiles
NCH = C // 128  # 6 contraction chunks
VSLOT = HPC * 65  # per-t-tile slot width in v_sb (64 v cols + 1 ones) * 3 heads
NCHUNK = 4  # reduce-scatter chunks
CH_ROWS = T // NCHUNK  # 1024
SH_ROWS = CH_ROWS // 4  # 256 rows per core per chunk

_CACHE = {}


def _build():
    if "nc" in _CACHE:
        return _CACHE["nc"]

    from concourse import bacc, tile, mybir

    dt = mybir.dt
    ActFn = mybir.ActivationFunctionType
    Alu = mybir.AluOpType

    nc = bacc.Bacc("TRN2", target_bir_lowering=False, debug=False,
                   num_devices=N_CORES)

    x_in = nc.dram_tensor("x", [T, C], dt.float32, kind="ExternalInput")
    wqk_in = nc.dram_tensor("wqk", [C, HPC * 128], dt.bfloat16, kind="ExternalInput")
    wv_in = nc.dram_tensor("wv", [C, HPC * 64], dt.bfloat16, kind="ExternalInput")
    wp_in = nc.dram_tensor("wp", [65, HPC * C], dt.bfloat16, kind="ExternalInput")
    bqk_in = nc.dram_tensor("bqk", [128, HPC], dt.float32, kind="ExternalInput")
    bv_in = nc.dram_tensor("bv", [1, HPC * 64], dt.bfloat16, kind="ExternalInput")
    ident_in = nc.dram_tensor("ident", [128, 128], dt.bfloat16, kind="ExternalInput")
    out_ext = nc.dram_tensor("out", [NCHUNK * SH_ROWS, C], dt.float32,
                             kind="ExternalOutput")

    groups = [[0, 1, 2, 3], [4, 5, 6, 7]]

    with tile.TileContext(nc) as tc:
        with (
            tc.tile_pool(name="persist", bufs=1) as pp,
            tc.tile_pool(name="dram", bufs=1, space="DRAM") as dp,
        ):
            # persistent SBUF tensors (single-slot pool tiles)
            xT = pp.tile([128, NCH * T], dt.bfloat16, tag="xT")
            qT2 = pp.tile([128, T], dt.bfloat16, tag="qT2")
            kT2 = pp.tile([128, T], dt.bfloat16, tag="kT2")
            qT3 = pp.tile([64, T], dt.bfloat16, tag="qT3")
            kT3 = pp.tile([64, T], dt.bfloat16, tag="kT3")
            vsb = pp.tile([128, NTT * VSLOT], dt.bfloat16, tag="vsb")
            wqk = pp.tile([128, NCH * HPC * 128], dt.bfloat16, tag="wqk")
            wv = pp.tile([128, NCH * HPC * 64], dt.bfloat16, tag="wv")
            wp = pp.tile([65, HPC * C], dt.bfloat16, tag="wp")
            bqk = pp.tile([128, HPC], dt.float32, tag="bqk")
            bv = pp.tile([1, HPC * 64], dt.bfloat16, tag="bv")
            ones = pp.tile([1, 128], dt.bfloat16, tag="ones")
            ident = pp.tile([128, 128], dt.bfloat16, tag="ident")
            yT = pp.tile([65, HPC * TB], dt.bfloat16, tag="yT")

            xbf_d = dp.tile([T, C], dt.bfloat16, tag="xbf_d")
            cc_in = dp.tile([T, C], dt.float32, tag="cc_in")
            cc_out = dp.tile([NCHUNK * SH_ROWS, C], dt.float32, tag="cc_out")

            # ---- load weights / constants ----
            nc.sync.dma_start(ident[:], ident_in.ap()[:])
            nc.sync.dma_start(bqk[:], bqk_in.ap()[:])
            nc.sync.dma_start(bv[:], bv_in.ap()[:])
            nc.vector.memset(ones[:], 1.0)
            nc.vector.memset(yT[64:65, :], 1.0)
            # wqk layout: chunk ci at [ci*HPC*128 : (ci+1)*HPC*128], head h at +h*128
            for ci in range(NCH):
                nc.sync.dma_start(wqk[:, ci * HPC * 128:(ci + 1) * HPC * 128],
                                  wqk_in.ap()[ci * 128:(ci + 1) * 128, :])
                nc.sync.dma_start(wv[:, ci * HPC * 64:(ci + 1) * HPC * 64],
                                  wv_in.ap()[ci * 128:(ci + 1) * 128, :])
            nc.sync.dma_start(wp[:], wp_in.ap()[:])

            with (
                tc.tile_pool(name="stage", bufs=3) as sp,
                tc.tile_pool(name="psA", bufs=2, space="PSUM") as psA,
            ):
                # ---- phase B: x -> bf16 -> DRAM -> xbar-transpose -> xT ----
                for j in range(NTT):
                    xf = sp.tile([128, C], dt.float32, tag="xf")
                    nc.sync.dma_start(xf[:], x_in.ap()[j * 128:(j + 1) * 128, :])
                    xb = sp.tile([128, C], dt.bfloat16, tag="xb")
                    nc.vector.tensor_copy(xb[:], xf[:])
                    nc.sync.dma_start(xbf_d[j * 128:(j + 1) * 128, :], xb[:])
                    if j % 8 == 7:
                        q0 = (j // 8) * 1024
                        for ci in range(NCH):
                            nc.scalar.dma_start_transpose(
                                xT[:, ci * T + q0: ci * T + q0 + 1024],
                                xbf_d[q0:q0 + 1024, ci * 128:(ci + 1) * 128])

                # ---- phase C: V (natural layout) for all 3 heads ----
                for j in range(NTT):
                    pv = psA.tile([128, HPC * 64], dt.float32, tag="pv")
                    for ci in range(NCH):
                        nc.tensor.matmul(
                            pv[:], xT[:, ci * T + j * 128: ci * T + (j + 1) * 128],
                            wv[:, ci * HPC * 64:(ci + 1) * HPC * 64],
                            start=(ci == 0), stop=False)
                    nc.tensor.matmul(pv[:], ones[:], bv[:], start=False, stop=True)
                    base = j * VSLOT
                    for hh in range(HPC):
                        nc.vector.tensor_copy(
                            vsb[:, base + hh * 65: base + hh * 65 + 64],
                            pv[:, hh * 64:(hh + 1) * 64])
                        nc.gpsimd.memset(vsb[:, base + hh * 65 + 64: base + hh * 65 + 65], 1.0)

                # ---- phase D: Q^T / K^T per head ----
                for hh in range(HPC):
                    for tb in range(NQB):
                        pq = psA.tile([128, TB], dt.float32, tag="pq")
                        for ci in range(NCH):
                            nc.tensor.matmul(
                                pq[:],
                                wqk[:, ci * HPC * 128 + hh * 128: ci * HPC * 128 + (hh + 1) * 128],
                                xT[:, ci * T + tb * TB: ci * T + (tb + 1) * TB],
                                start=(ci == 0), stop=(ci == NCH - 1))
                        st = sp.tile([128, TB], dt.bfloat16, tag="qkst")
                        nc.vector.tensor_scalar_add(st[:], pq[:], bqk[:, hh:hh + 1])
                        tsl = slice(tb * TB, (tb + 1) * TB)
                        if hh < 2:
                            nc.sync.dma_start(qT2[hh * 64:(hh + 1) * 64, tsl], st[0:64, :])
                            nc.sync.dma_start(kT2[hh * 64:(hh + 1) * 64, tsl], st[64:128, :])
                        else:
                            nc.sync.dma_start(qT3[:, tsl], st[0:64, :])
                            nc.sync.dma_start(kT3[:, tsl], st[64:128, :])

            # ---- phase E: attention + projection + chunked reduce-scatter ----
            with (
                tc.tile_pool(name="ptp", bufs=4) as ptp,
                tc.tile_pool(name="misc", bufs=2) as mp,
                tc.tile_pool(name="outp", bufs=3) as op_,
                tc.tile_pool(name="ps_s", bufs=2, space="PSUM") as ps_s,
                tc.tile_pool(name="ps_y", bufs=2, space="PSUM") as ps_y,
                tc.tile_pool(name="ps_p", bufs=1, space="PSUM") as ps_p,
            ):
                for qb in range(NQB):
                    n_kv = 4 * (qb + 1)
                    diag0 = 4 * qb  # first diagonal kv tile index
                    qsl = slice(qb * TB, (qb + 1) * TB)

                    # heads 0+1: row-packed concurrent matmuls
                    py0 = ps_y.tile([128, TB], dt.float32, tag="py")
                    py1 = ps_y.tile([128, TB], dt.float32, tag="py")
                    for j in range(n_kv):
                        ss = ps_s.tile([128, 2 * TB], dt.float32, tag="ss")
                        pt = ptp.tile([128, 2 * TB], dt.bfloat16, tag="pt")
                        jsl = slice(j * KT, (j + 1) * KT)
                        nc.tensor.matmul(ss[:, 0:TB], kT2[0:64, jsl],
                                         qT2[0:64, qsl], start=True, stop=True)
                        nc.tensor.matmul(ss[:, TB:2 * TB], kT2[64:128, jsl],
                                         qT2[64:128, qsl], start=True, stop=True)
                        if j < diag0:
                            nc.scalar.activation(pt[:], ss[:], ActFn.Exp,
                                                 scale=float(HD) ** -0.5)
                        else:
                            k0 = (j - diag0) * KT
                            for u in range(2):
                                sl = slice(u * TB + k0, (u + 1) * TB)
                                nc.scalar.activation(pt[:, sl], ss[:, sl],
                                                     ActFn.Exp,
                                                     scale=float(HD) ** -0.5)
                                if k0 > 0:
                                    nc.gpsimd.memset(pt[:, u * TB: u * TB + k0], 0.0)
                                nc.gpsimd.affine_select(
                                    pt[:, u * TB + k0: u * TB + k0 + KT],
                                    pt[:, u * TB + k0: u * TB + k0 + KT],
                                    pattern=[[1, KT]], base=0,
                                    channel_multiplier=-1,
                                    compare_op=Alu.is_ge, fill=0.0)
                        nc.tensor.matmul(py0[0:65, :],
                                         vsb[:, j * VSLOT: j * VSLOT + 65],
                                         pt[:, 0:TB],
                                         start=(j == 0), stop=(j == n_kv - 1))
                        nc.tensor.matmul(py1[0:65, :],
                                         vsb[:, j * VSLOT + 65: j * VSLOT + 130],
                                         pt[:, TB:2 * TB],
                                         start=(j == 0), stop=(j == n_kv - 1))
                    for hh, py in ((0, py0), (1, py1)):
                        rec = mp.tile([65, TB], dt.float32, tag="rec")
                        nc.vector.reciprocal(rec[64:65, :], py[64:65, :])
                        sbb = mp.tile([64, TB], dt.float32, tag="sbb")
                        nc.gpsimd.dma_start(
                            sbb[:],
                            rec[64:65, :].unsqueeze(1).broadcast_to([1, 64, TB]))
                        nc.vector.tensor_tensor(
                            yT[0:64, hh * TB:(hh + 1) * TB], py[0:64, :], sbb[:],
                            op=Alu.mult)

                    # head 2: solo, two kv tiles per pass
                    py = ps_y.tile([128, TB], dt.float32, tag="py")
                    for m in range(n_kv // 2):
                        ss = ps_s.tile([128, 2 * TB], dt.float32, tag="ss")
                        pt = ptp.tile([128, 2 * TB], dt.bfloat16, tag="pt")
                        for u in range(2):
                            j = 2 * m + u
                            nc.tensor.matmul(
                                ss[:, u * TB:(u + 1) * TB],
                                kT3[:, j * KT:(j + 1) * KT],
                                qT3[:, qsl], start=True, stop=True)
                        if 2 * m + 1 < diag0:
                            nc.scalar.activation(pt[:], ss[:], ActFn.Exp,
                                                 scale=float(HD) ** -0.5)
                        else:
                            for u in range(2):
                                j = 2 * m + u
                                k0 = (j - diag0) * KT
                                sl = slice(u * TB + k0, (u + 1) * TB)
                                nc.scalar.activation(pt[:, sl], ss[:, sl],
                                                     ActFn.Exp,
                                                     scale=float(HD) ** -0.5)
                                if k0 > 0:
                                    nc.gpsimd.memset(pt[:, u * TB: u * TB + k0], 0.0)
                                nc.gpsimd.affine_select(
                                    pt[:, u * TB + k0: u * TB + k0 + KT],
                                    pt[:, u * TB + k0: u * TB + k0 + KT],
                                    pattern=[[1, KT]], base=0,
                                    channel_multiplier=-1,
                                    compare_op=Alu.is_ge, fill=0.0)
                        for u in range(2):
                            j = 2 * m + u
                            nc.tensor.matmul(
                                py[0:65, :],
                                vsb[:, j * VSLOT + 2 * 65: j * VSLOT + 3 * 65],
                                pt[:, u * TB:(u + 1) * TB],
                                start=(j == 0), stop=(j == n_kv - 1))
                    rec = mp.tile([65, TB], dt.float32, tag="rec")
                    nc.vector.reciprocal(rec[64:65, :], py[64:65, :])
                    sbb = mp.tile([64, TB], dt.float32, tag="sbb")
                    nc.gpsimd.dma_start(
                        sbb[:],
                        rec[64:65, :].unsqueeze(1).broadcast_to([1, 64, TB]))
                    nc.vector.tensor_tensor(
                        yT[0:64, 2 * TB:3 * TB], py[0:64, :], sbb[:],
                        op=Alu.mult)

                    # projection for this q block
                    for qs in range(4):
                        t0 = qb * TB + qs * 128
                        pp_ = ps_p.tile([128, 1024], dt.float32, tag="ppj")
                        for half in range(2):
                            ob = half * 512  # bank-aligned offset inside psum tile
                            for hh in range(HPC):
                                kk = 65 if hh == 0 else 64  # head-0 row 64 = ones x bias row
                                nc.tensor.matmul(
                                    pp_[:, ob: ob + 384],
                                    yT[0:kk, hh * TB + qs * 128: hh * TB + (qs + 1) * 128],
                                    wp[0:kk, hh * C + half * 384: hh * C + half * 384 + 384],
                                    start=(hh == 0), stop=(hh == HPC - 1))
                        osb = op_.tile([128, C], dt.float32, tag="osb")
                        nc.vector.tensor_copy(osb[:, 0:384], pp_[:, 0:384])
                        nc.vector.tensor_copy(osb[:, 384:768], pp_[:, 512:896])
                        nc.scalar.dma_start(cc_in[t0:t0 + 128, :], osb[:])

                    # chunked reduce-scatter every 2 q blocks (1024 rows)
                    if qb % 2 == 1:
                        ch = qb // 2
                        r0 = ch * CH_ROWS
                        s0 = ch * SH_ROWS
                        nc.gpsimd.collective_compute(
                            "ReduceScatter", Alu.add, replica_groups=groups,
                            ins=[cc_in[r0:r0 + CH_ROWS, :]],
                            outs=[cc_out[s0:s0 + SH_ROWS, :]])
                        nc.sync.dma_start(out_ext.ap()[s0:s0 + SH_ROWS, :],
                                          cc_out[s0:s0 + SH_ROWS, :])

    nc.compile()
    _CACHE["nc"] = nc
    return nc


def _prep_core_inputs(x, w_attn, b_attn, w_proj, b_proj):
    """Host-side sharding: returns list of 8 input dicts."""
    import ml_dtypes

    bf16 = ml_dtypes.bfloat16
    ident = np.eye(128, dtype=bf16)
    in_maps = []
    for core in range(N_CORES):
        b = core // 4
        h0 = HPC * (core % 4)
        # wqk: per head [q cols | k cols] -> [768, 3*128]
        wqk = np.empty((C, HPC * 128), np.float32)
        bqk = np.empty((128, HPC), np.float32)
        wv = np.empty((C, HPC * 64), np.float32)
        bv = np.empty((1, HPC * 64), np.float32)
        wp = np.zeros((65, HPC * C), np.float32)
        wp[64, 0:C] = b_proj / 4.0
        for hh in range(HPC):
            h = h0 + hh
            wqk[:, hh * 128: hh * 128 + 64] = w_attn[:, h * HD:(h + 1) * HD]
            wqk[:, hh * 128 + 64: hh * 128 + 128] = w_attn[:, C + h * HD: C + (h + 1) * HD]
            bqk[0:64, hh] = b_attn[h * HD:(h + 1) * HD]
            bqk[64:128, hh] = b_attn[C + h * HD: C + (h + 1) * HD]
            wv[:, hh * 64:(hh + 1) * 64] = w_attn[:, 2 * C + h * HD: 2 * C + (h + 1) * HD]
            bv[0, hh * 64:(hh + 1) * 64] = b_attn[2 * C + h * HD: 2 * C + (h + 1) * HD]
            wp[0:64, hh * C:(hh + 1) * C] = w_proj[h * HD:(h + 1) * HD, :]
        in_maps.append({
            "x": np.ascontiguousarray(x[b], np.float32),
            "wqk": wqk.astype(bf16),
            "wv": wv.astype(bf16),
            "wp": wp.astype(bf16),
            "bqk": bqk,
            "bv": bv.astype(bf16),
            "ident": ident,
        })
    return in_maps


def kernel(x, w_attn, b_attn, w_proj, b_proj, _trace=False, _trace_kwargs=None):
    x = np.asarray(x, np.float32)
    w_attn = np.asarray(w_attn, np.float32)
    b_attn = np.asarray(b_attn, np.float32)
    w_proj = np.asarray(w_proj, np.float32)
    b_proj = np.asarray(b_proj, np.float32)

    nc = _build()
    from concourse.bass_utils import run_bass_kernel_spmd

    in_maps = _prep_core_inputs(x, w_attn, b_attn, w_proj, b_proj)
    kw = dict(_trace_kwargs or {})
    res = run_bass_kernel_spmd(nc, in_maps, core_ids=list(range(N_CORES)),
                               trace=_trace, **kw)
    # reassemble: core 4*b + r holds, for each chunk c, global rows
    # c*1024 + r*256 .. +256 in its out[c*256:(c+1)*256]
    out = np.empty((B, T, C), np.float32)
    for b in range(B):
        for r in range(4):
            o = res.results[4 * b + r]["out"]
            for ch in range(NCHUNK):
                g0 = ch * CH_ROWS + r * SH_ROWS
                out[b, g0:g0 + SH_ROWS] = o[ch * SH_ROWS:(ch + 1) * SH_ROWS]
    if _trace:
        kernel.last_results = res
    return out
